# revision 38
# baseline (speedup 1.0000x reference)
"""Causal multi-head attention (b=2, n=2048, d=768, 12 heads) on 8 TRN2 NeuronCores.

Sharding: batch x head-group. Core c handles batch c//4 and heads 3*(c%4) .. 3*(c%4)+2.
Each core gets xT = x[b].T plus W.T column slices for its 3 heads, computes the
unnormalized attention output (transposed) plus softmax denominators; the host
divides, transposes, and concatenates slabs into the full [2, 2048, 768].

v2 design (measured-on-hw rationale):
  - ACT exp is the limiting engine (~62us of back-to-back exps). The whole
    kernel is emitted as one software-pipelined stream over (span, head-loop,
    j-tile) stages: SC(i+1) is emitted BEFORE AV(i) so the exp for stage i+1
    can start the moment exp(i) retires; projection matmuls for later spans
    are injected as PE filler between SC and AV so the Tensor engine's
    in-order queue never blocks ACT.
  - fp16 operands for x/W/q/k (10-bit mantissa keeps score error ~1e-2),
    bf16 for p=exp(s) (needs e^66 range) and v. PSUM stays f32. Measured:
    512-free matmuls cost ~217-228ns for ALL of fp16/bf16/f32r (the PE is
    stream-bound at ~0.44ns/row; chaining, lhsT reuse irrelevant), so
    16-bit only buys DMA/SBUF/DVE bandwidth, not PE time.
  - v is projected directly into natural [keys, hd] layout with x-stationary
    matmuls (lhsT = xT chunk), eliminating all PE transposes and 2 of the 5
    projection M-chunks.
  - no fp32 warmup burn; input DMAs are chunked (weights on the gpsimd SWDGE
    queue, x on the sync queue) so the first projection starts ~1us after the
    runtime's fixed ~7us engine-init barrier.

Per-core algorithm (transposed so softmax denominators ride the AV matmul):
  qT/kT = (W.T slice).T @ xT          per 512-col span, 3 M-chunks (q01|k01|q2k2)
  v_nat[128 keys, 64+1] = xT_chunk.T @ Wv  (+ ones column -> denominator)
  per span s, per j-tile (pair loop h0,h1 then solo loop h2):
    sT[j] = kTz.T @ qT[:, n0:512]     (K=128, zero-padded head halves)
    p = exp(sT) unshifted, bf16; diagonal 128-blocks masked by 0/1 triangle
    av[0:65, span] += v_nat[j].T @ p  (row 64 accumulates denom)
  av -> SBUF f32 -> DRAM; host computes (av[0:64]/av[64]).T per head.
"""
import sys

if "/opt/trn_rl_repo" not in sys.path:
    sys.path.insert(0, "/opt/trn_rl_repo")

from collections import deque
from contextlib import ExitStack

import numpy as np

import concourse.bass as bass
import concourse.tile as tile
from concourse import bacc, mybir, bass_utils
from concourse.masks import make_upper_triangular

F32 = mybir.dt.float32
FP16 = mybir.dt.float16
BF16 = mybir.dt.bfloat16

P = 128
SPAN = 512
HD = 64

B, N, D, NH = 2, 2048, 768, 12
HL = 3                       # heads per core
DL = HL * HD                 # 192
N_CORES = 8
KT = D // P                  # 6 contraction chunks
NS = N // SPAN               # 4 spans
NT = N // P                  # 16 j-tiles
CPS = SPAN // P              # 4 chunks per span
WQK = 3 * P                  # 384 packed q/k weight cols per kt


def _build(nc, tc):
    xt = nc.dram_tensor("xt", [P, NS * KT * SPAN], FP16,
                        kind="ExternalInput").ap()
    wqk = nc.dram_tensor("wqk", [P, KT * WQK], FP16,
                         kind="ExternalInput").ap()
    wv = nc.dram_tensor("wv", [P, KT * DL], FP16, kind="ExternalInput").ap()
    o = nc.dram_tensor("o", [HL * (HD + 1), N], F32, kind="ExternalOutput").ap()

    with ExitStack() as ctx:
        pool = lambda name, bufs, **kw: ctx.enter_context(
            tc.tile_pool(name=name, bufs=bufs, **kw))
        const_pool = pool("const", 1)
        xpool = pool("x", NS)
        wpool = pool("w", 2)
        qk_pool = pool("qk", 2 * NS)
        kz_pool = pool("kz", HL * NS)
        vnat_pool = pool("vnat", 2)
        ppool = pool("p", 4)
        osb_pool = pool("osb", 3)
        ps_proj = pool("ps_proj", 2, space="PSUM")   # 2 banks
        ps_sc = pool("ps_sc", 2, space="PSUM")       # [128,1024] x2 = 4 banks
        ps_av = pool("ps_av", 2, space="PSUM")       # 2 banks

        wtile = const_pool.tile([P, SPAN], FP16)
        trimask = const_pool.tile([P, P], BF16)

        x_tiles = [xpool.tile([P, KT * SPAN], FP16, tag="x", name=f"x{i}")
                   for i in range(NS)]
        wqk_t = wpool.tile([P, KT * WQK], FP16, tag="wqk")
        wv_t = wpool.tile([P, KT * DL], FP16, tag="wv")

        qT01 = [qk_pool.tile([P, SPAN], FP16, tag="q01", name=f"q01_{i}")
                for i in range(NS)]
        qT2z = [qk_pool.tile([P, SPAN], FP16, tag="q2z", name=f"q2z_{i}")
                for i in range(NS)]
        kTz = [[kz_pool.tile([P, SPAN], FP16, tag="kz", name=f"kz_{h}_{i}")
                for i in range(NS)] for h in range(HL)]

        v_nat01 = vnat_pool.tile([P, NT * 2 * (HD + 1)], BF16, tag="vnat01")
        v_nat2 = vnat_pool.tile([P, NT * (HD + 1)], BF16, tag="vnat2")
        c01 = v_nat01[:].rearrange("p (t c) -> p t c", c=HD + 1)[:, :, HD]
        c2 = v_nat2[:].rearrange("p (t c) -> p t c", c=HD + 1)[:, :, HD]

        # ---- init: span-0 pads at the head (DVE idle there); later spans'
        # pads are emitted inside the filler generators so they don't
        # serialize ahead of the first projection casts on the in-order
        # DVE queue.
        nc.gpsimd.memset(wtile[:], 0.5)
        make_upper_triangular(nc, trimask[:], val=1.0, diag=True)

        def zfill(ap):
            nc.vector.memset(ap, 0.0)

        zfill(kTz[0][0][HD:P, :])
        zfill(kTz[1][0][0:HD, :])
        zfill(kTz[2][0][HD:P, :])
        zfill(qT2z[0][HD:P, :])
        nc.vector.memset(c01, 1.0)
        nc.vector.memset(c2, 1.0)

        # ---- DMAs: all on the sync HWDGE queue (the gpsimd SWDGE path runs
        # at ~50GB/s vs ~1TB/s here); full-span transfers (big contiguous
        # per-partition runs move ~10x faster than 1KB chunks); weights
        # first since the first projection needs wqk + x0.
        nc.sync.dma_start(wqk_t[:], wqk[:])
        hx = KT * SPAN // 2
        nc.sync.dma_start(x_tiles[0][:, 0:hx], xt[:, 0:hx])
        nc.sync.dma_start(x_tiles[0][:, hx:2 * hx], xt[:, hx:2 * hx])
        nc.sync.dma_start(wv_t[:], wv[:])
        for ns in range(1, NS):
            w = KT * SPAN
            nc.sync.dma_start(x_tiles[ns][:], xt[:, ns * w:(ns + 1) * w])

        # ---- warm matmuls: PE runs at ~1.2GHz until ~3us of continuous
        # execution, and drops back when idle; keep it busy from engine-init
        # until x0 lands (~13us: ~360GB/s aggregate DMA, 1.4MB critical) so
        # the ramp to 2.4GHz burns DMA-wait time, not projection time.
        wp = ps_proj.tile([P, SPAN], F32, tag="ps_proj", name="warm")
        for _ in range(24):
            nc.tensor.matmul(wp[:, 0:256], wtile[:, 0:P], wtile[:, 0:256],
                             start=True, stop=True)

        def x_slice(ns, kt):
            return x_tiles[ns][:, kt * SPAN:(kt + 1) * SPAN]

        # ---- projection filler groups (generators; 1 yield = ~1 PE matmul)
        def gen_qk01(ns):
            # q01 and k01 interleaved by kt so span-0's matmuls track the
            # chunked x0 DMA instead of serializing behind it
            ptq = ps_proj.tile([P, SPAN], F32, tag="ps_proj")
            ptk = ps_proj.tile([P, SPAN], F32, tag="ps_proj")
            for kt in range(KT):
                nc.tensor.matmul(
                    ptq[:], wqk_t[:, kt * WQK:kt * WQK + P],
                    x_slice(ns, kt), start=(kt == 0), stop=(kt == KT - 1))
                yield
                nc.tensor.matmul(
                    ptk[:], wqk_t[:, kt * WQK + P:kt * WQK + 2 * P],
                    x_slice(ns, kt), start=(kt == 0), stop=(kt == KT - 1))
                yield
            nc.vector.tensor_copy(qT01[ns][:], ptq[:])
            yield
            if ns == 0:
                # ACT is idle before the first exp: run the k01 copies there,
                # in parallel with the q01 cast on DVE, to start SC(0) sooner
                nc.scalar.activation(kTz[0][ns][0:HD, :], ptk[0:HD, :],
                                     mybir.ActivationFunctionType.Copy)
                nc.scalar.activation(kTz[1][ns][HD:P, :], ptk[HD:P, :],
                                     mybir.ActivationFunctionType.Copy)
            else:
                nc.vector.tensor_copy(kTz[0][ns][0:HD, :], ptk[0:HD, :])
                nc.vector.tensor_copy(kTz[1][ns][HD:P, :], ptk[HD:P, :])
                zfill(kTz[0][ns][HD:P, :])
                zfill(kTz[1][ns][0:HD, :])
            yield

        def gen_q2k2(ns):
            pt = ps_proj.tile([P, SPAN], F32, tag="ps_proj")
            for kt in range(KT):
                nc.tensor.matmul(
                    pt[:],
                    wqk_t[:, kt * WQK + 2 * P:kt * WQK + 3 * P],
                    x_slice(ns, kt),
                    start=(kt == 0), stop=(kt == KT - 1))
                yield
            nc.vector.tensor_copy(qT2z[ns][0:HD, :], pt[0:HD, :])
            nc.vector.tensor_copy(kTz[2][ns][0:HD, :], pt[HD:P, :])
            if ns > 0:
                zfill(qT2z[ns][HD:P, :])
                zfill(kTz[2][ns][HD:P, :])
            yield

        def gen_vnat(ns, c):
            jt = ns * CPS + c
            pv = ps_proj.tile([P, SPAN], F32, tag="ps_proj")
            for kt in range(KT):
                nc.tensor.matmul(
                    pv[:, 0:DL],
                    x_tiles[ns][:, kt * SPAN + c * P:kt * SPAN + (c + 1) * P],
                    wv_t[:, kt * DL:(kt + 1) * DL],
                    start=(kt == 0), stop=(kt == KT - 1))
                yield
            nc.vector.tensor_copy(
                v_nat01[:].rearrange("p (t c) -> p t c", c=HD + 1)[
                    :, 2 * jt:2 * jt + 2, 0:HD],
                pv[:, 0:2 * HD].rearrange("p (t c) -> p t c", c=HD))
            nc.vector.tensor_copy(
                v_nat2[:, jt * (HD + 1):jt * (HD + 1) + HD], pv[:, 2 * HD:DL])
            yield

        # filler queue: (seq, generator) in span order
        fillers = deque()
        seq_counter = [0]
        SEQ_K01 = {}
        SEQ_Q2K2 = {}
        SEQ_VNAT = {}

        def add_group(gen, reg=None, key=None):
            s = seq_counter[0]
            seq_counter[0] += 1
            fillers.append((s, gen))
            if reg is not None:
                reg[key] = s

        for ns in range(NS):
            add_group(gen_qk01(ns), SEQ_K01, ns)
            add_group(gen_q2k2(ns), SEQ_Q2K2, ns)
            for c in range(CPS):
                add_group(gen_vnat(ns, c), SEQ_VNAT, ns * CPS + c)

        def flush_until(seq):
            while fillers and fillers[0][0] <= seq:
                s, g = fillers[0]
                for _ in g:
                    pass
                fillers.popleft()

        def pop_fillers(n):
            while n > 0 and fillers:
                s, g = fillers[0]
                try:
                    next(g)
                    n -= 1
                except StopIteration:
                    fillers.popleft()

        # ---- attention stages: h2 j-tiles are paired so two score tiles
        # share one exp instruction (halves h2 ACT instruction overhead)
        stages = []
        for s in range(NS):
            njt = CPS * (s + 1)
            stages += [("pair", s, [jt], njt) for jt in range(njt)]
            stages += [("h2", s, [2 * k, 2 * k + 1], njt)
                       for k in range(njt // 2)]

        ST = {}      # stage idx -> dict(sc=, p=)
        AVT = {}     # (kind, s) -> av tiles

        def vnat_lhsT(h, jt):
            if h < 2:
                b = jt * 2 * (HD + 1) + h * (HD + 1)
                return v_nat01[:, b:b + HD + 1]
            b = jt * (HD + 1)
            return v_nat2[:, b:b + HD + 1]

        def emit_sc(i):
            kind, s, jts, njt = stages[i]
            if kind == "pair":
                flush_until(SEQ_K01[s])
            else:
                flush_until(SEQ_Q2K2[s])
            sc = ps_sc.tile([P, 2 * SPAN], F32, tag="ps_sc")
            if kind == "pair":
                jt = jts[0]
                n0 = max(jt - CPS * s, 0) * P
                ns_k, ck = jt // CPS, jt % CPS
                nc.tensor.matmul(
                    sc[:, n0:SPAN],
                    kTz[0][ns_k][:, ck * P:(ck + 1) * P],
                    qT01[s][:, n0:SPAN], start=True, stop=True)
                nc.tensor.matmul(
                    sc[:, SPAN + n0:2 * SPAN],
                    kTz[1][ns_k][:, ck * P:(ck + 1) * P],
                    qT01[s][:, n0:SPAN], start=True, stop=True)
            else:
                for idx, jt in enumerate(jts):
                    n0 = max(jt - CPS * s, 0) * P
                    ns_k, ck = jt // CPS, jt % CPS
                    nc.tensor.matmul(
                        sc[:, idx * SPAN + n0:(idx + 1) * SPAN],
                        kTz[2][ns_k][:, ck * P:(ck + 1) * P],
                        qT2z[s][:, n0:SPAN], start=True, stop=True)
            ST[i] = {"sc": sc}

        def emit_exp(i):
            kind, s, jts, njt = stages[i]
            sc = ST[i]["sc"]
            p = ppool.tile([P, 2 * SPAN], BF16, tag="p")
            if kind == "pair":
                jt = jts[0]
                c_d = jt - CPS * s
                n0 = max(c_d, 0) * P
                if c_d < 0:
                    # both halves full: one contiguous 1D exp (cheaper AP)
                    nc.scalar.activation(p[:, 0:2 * SPAN], sc[:, 0:2 * SPAN],
                                         mybir.ActivationFunctionType.Exp)
                else:
                    sc3 = sc[:].rearrange("q (t c) -> q t c", c=SPAN)
                    p3 = p[:].rearrange("q (t c) -> q t c", c=SPAN)
                    nc.scalar.activation(p3[:, :, n0:SPAN], sc3[:, :, n0:SPAN],
                                         mybir.ActivationFunctionType.Exp)
                    nc.vector.tensor_mul(
                        p[:, n0:n0 + P], p[:, n0:n0 + P], trimask[:])
                    nc.vector.tensor_mul(
                        p[:, SPAN + n0:SPAN + n0 + P],
                        p[:, SPAN + n0:SPAN + n0 + P], trimask[:])
            else:
                # one exp across both paired j-tiles; any stale psum between
                # the two valid regions is exp'd but never read downstream
                n0a = max(jts[0] - CPS * s, 0) * P
                nc.scalar.activation(p[:, n0a:2 * SPAN], sc[:, n0a:2 * SPAN],
                                     mybir.ActivationFunctionType.Exp)
                for idx, jt in enumerate(jts):
                    c_d = jt - CPS * s
                    if c_d >= 0:
                        n0 = c_d * P
                        b = idx * SPAN + n0
                        nc.vector.tensor_mul(
                            p[:, b:b + P], p[:, b:b + P], trimask[:])
            ST[i]["p"] = p

        def finalize(s, h, av):
            ob = osb_pool.tile([HD + 1, SPAN], F32, tag="osb")
            nc.vector.tensor_copy(ob[:], av[:])
            nc.sync.dma_start(
                o[h * (HD + 1):(h + 1) * (HD + 1),
                  s * SPAN:(s + 1) * SPAN], ob[:])

        def emit_av(i):
            kind, s, jts, njt = stages[i]
            p = ST[i]["p"]
            flush_until(SEQ_VNAT[jts[-1]])
            if jts[0] == 0:
                if kind == "pair":
                    AVT[("pair", s)] = (
                        ps_av.tile([HD + 1, SPAN], F32, tag="ps_av",
                                   name=f"av0_{s}"),
                        ps_av.tile([HD + 1, SPAN], F32, tag="ps_av",
                                   name=f"av1_{s}"))
                else:
                    AVT[("h2", s)] = (
                        ps_av.tile([HD + 1, SPAN], F32, tag="ps_av",
                                   name=f"av2_{s}"),)
            if kind == "pair":
                jt = jts[0]
                n0 = max(jt - CPS * s, 0) * P
                start, stop = (jt == 0), (jt == njt - 1)
                av0, av1 = AVT[("pair", s)]
                nc.tensor.matmul(av0[:, n0:SPAN], vnat_lhsT(0, jt),
                                 p[:, n0:SPAN], start=start, stop=stop)
                nc.tensor.matmul(av1[:, n0:SPAN], vnat_lhsT(1, jt),
                                 p[:, SPAN + n0:2 * SPAN],
                                 start=start, stop=stop)
                if stop:
                    finalize(s, 0, av0)
                    finalize(s, 1, av1)
            else:
                (av2,) = AVT[("h2", s)]
                for idx, jt in enumerate(jts):
                    n0 = max(jt - CPS * s, 0) * P
                    nc.tensor.matmul(
                        av2[:, n0:SPAN], vnat_lhsT(2, jt),
                        p[:, idx * SPAN + n0:(idx + 1) * SPAN],
                        start=(jt == 0), stop=(jt == njt - 1))
                if jts[-1] == njt - 1:
                    finalize(s, 2, av2)
            del ST[i]

        # ---- software-pipelined emission: SC(i+1) before AV(i) ----
        emit_sc(0)
        for i in range(len(stages)):
            emit_exp(i)
            if i + 1 < len(stages):
                emit_sc(i + 1)
            pop_fillers(3 if stages[i][0] == "pair" else 4)
            emit_av(i)
        flush_until(seq_counter[0])


_NC_CACHE = {}


def _get_module():
    if "m" not in _NC_CACHE:
        nc = bacc.Bacc("TRN2", target_bir_lowering=False, debug=False)
        with tile.TileContext(nc) as tc:
            _build(nc, tc)
        nc.compile()
        _NC_CACHE["m"] = nc
    return _NC_CACHE["m"]


def _in_maps(x, Wq, Wk, Wv):
    maps = []
    xT = [np.ascontiguousarray(
        x[b].T.reshape(KT, P, NS, SPAN).transpose(1, 2, 0, 3).reshape(P, -1))
        .astype(np.float16) for b in range(B)]
    WqT, WkT, WvT = Wq.T, Wk.T, Wv.T
    for c in range(N_CORES):
        bc, g = divmod(c, N_CORES // B)
        s0 = g * DL
        wqk_cols = np.concatenate([
            WqT[:, s0:s0 + P], WkT[:, s0:s0 + P],
            WqT[:, s0 + P:s0 + DL], WkT[:, s0 + P:s0 + DL]], axis=1)
        wqk_pk = np.ascontiguousarray(
            wqk_cols.reshape(KT, P, WQK).transpose(1, 0, 2).reshape(P, -1)
        ).astype(np.float16)
        wv_pk = np.ascontiguousarray(
            WvT[:, s0:s0 + DL].reshape(KT, P, DL).transpose(1, 0, 2)
            .reshape(P, -1)).astype(np.float16)
        maps.append({
            "xt": xT[bc],
            "wqk": wqk_pk,
            "wv": wv_pk,
        })
    return maps


def kernel(x, Wq, Wk, Wv, _trace=False, _tmpdir=None, **_kw):
    x = np.asarray(x, dtype=np.float32)
    Wq = np.asarray(Wq, dtype=np.float32)
    Wk = np.asarray(Wk, dtype=np.float32)
    Wv = np.asarray(Wv, dtype=np.float32)
    assert x.shape == (B, N, D) and Wq.shape == (D, D)

    nc = _get_module()
    res = bass_utils.run_bass_kernel_spmd(
        nc, _in_maps(x, Wq, Wk, Wv), core_ids=list(range(N_CORES)),
        trace=_trace, tmpdir=_tmpdir)
    out = np.empty((B, N, D), np.float32)
    for c in range(N_CORES):
        bc, g = divmod(c, N_CORES // B)
        oT = res.results[c]["o"].astype(np.float64)
        for h in range(HL):
            blk = oT[h * (HD + 1):h * (HD + 1) + HD, :]
            den = oT[h * (HD + 1) + HD, :]
            out[bc, :, g * DL + h * HD:g * DL + (h + 1) * HD] = \
                (blk / den).T.astype(np.float32)
    if _trace:
        return out, res
    return out
